# revision 19
# baseline (speedup 1.0000x reference)
"""GQA transformer block on 8 TRN2 cores — query-sharded (no reduce-scatter).

Sharding: core = (b, r), b = batch (2), r = query-quarter (4).  Core (b, r)
owns query tiles TILES[r] = sorted({r, 7-r, 8+r, 15-r}) of batch b (causally
balanced: every core sees exactly 34 key-tiles of attention work) and computes
ALL 16 q-heads for its 512 queries.  K/V are projected shard-wise and
exchanged with two small AllGathers (K^T-form, V-natural-form); no output
collective at all.

Attention runs in S^T orientation (keys on partitions, queries on free dim).
The two heads of a pair sit at base partitions 0/64 so their contract-64
score matmuls land on different PE row-groups and run concurrently on HW.
Head B's V-stationary is the same buffer shifted 2 cols ([V|1|1|V] layout) so
its V-sums land at psum rows 64..127 — the partitions its Wo-stationary
needs — with its softmax denominator at row 63; head A uses [V|1] (Z row 64).

Causality is rank-uniform: key tile g (processed in AllGather order PERM)
covers query cols [128*(g//4), 512); only the first 128-col slot is
rank-uncertain, and it gets a host-supplied additive mask M fused into a
Schraudolph exp on DVE: int16(x*A + M) bitcast to bf16 is ~exp(x/8) where
M = B, and ~0 (denormal) where M = -32750.  The rest of each block gets real
exp on ACT / bit-exp on DVE, alternating to balance the engines.
"""

import os
import sys
from contextlib import ExitStack

for _p in ("/opt/trn_rl_repo", "/root/.axon_site/_ro/trn_rl_repo"):
    if os.path.isdir(_p) and _p not in sys.path:
        sys.path.insert(0, _p)

import numpy as np
import ml_dtypes

import concourse.bass as bass
import concourse.bacc as bacc
import concourse.tile as tile
from concourse import mybir
from concourse.bass_utils import run_bass_kernel_spmd
from concourse.masks import make_identity

B, N, D = 2, 2048, 1024
HQ, HKV, HD = 16, 4, 64
G = HQ // HKV
EPS = 1e-5
P = 128
NT = N // P          # 16 token tiles
DC = D // P          # 8 d-chunks
NP = HQ // 2         # 8 head pairs
F32 = mybir.dt.float32
F32R = mybir.dt.float32r
BF16 = mybir.dt.bfloat16
I16 = mybir.dt.int16
AF = mybir.ActivationFunctionType
ALU = mybir.AluOpType
SCALE = 1.0 / np.sqrt(HD)
RG = [[0, 1, 2, 3], [4, 5, 6, 7]]
BF = ml_dtypes.bfloat16

TILES = [sorted([r, 7 - r, 8 + r, 15 - r]) for r in range(4)]
PERM = [g for r in range(4) for g in TILES[r]]   # key-tile processing order

# Schraudolph bf16 exp: bf16(int16(x * A + B)) ~ exp(x); SCALE folded into A.
SCH_A = (128.0 / np.log(2.0)) * SCALE
SCH_B = 16256.4          # 127*128, +0.4 rounding slack
# positions whose clean-exp runs on DVE (bit-exp); rest on ACT.  w per pos is
# 512-128*(PERM[pos]//4); give DVE the narrow blocks to balance the engines.
DVE_POS = frozenset(p for p in range(16) if PERM[p] // 4 >= 2)


def build_program(skip_bias=False, skip_g2b2=False):
    nc = bacc.Bacc(None, target_bir_lowering=False, num_devices=8)
    xs = nc.declare_dram_parameter("xs", [4, P, D], F32, isOutput=False)
    wq = nc.declare_dram_parameter("wq", [P, DC, HQ * HD], BF16, isOutput=False)
    wkv = nc.declare_dram_parameter("wkv", [P, DC, 2 * HKV * HD], BF16,
                                    isOutput=False)
    wo = nc.declare_dram_parameter("wo", [P, NP, D], BF16, isOutput=False)
    brow = nc.declare_dram_parameter("brow", [1, HQ * HD + 2 * HKV * HD], BF16,
                                     isOutput=False)
    g2b2 = nc.declare_dram_parameter("g2b2", [2, D], F32, isOutput=False)
    msk = nc.declare_dram_parameter("msk", [P, NT, 2, P], BF16, isOutput=False)
    y = nc.declare_dram_parameter("y", [4, P, D], F32, isOutput=True)

    with tile.TileContext(nc) as tc, ExitStack() as ctx:
        const = ctx.enter_context(tc.tile_pool(name="const", bufs=1))
        big = ctx.enter_context(tc.tile_pool(name="big", bufs=1))
        work = ctx.enter_context(tc.tile_pool(name="work", bufs=3))
        stp = ctx.enter_context(tc.tile_pool(name="stats", bufs=4))
        pssc = ctx.enter_context(tc.tile_pool(name="pssc", bufs=2, space="PSUM"))
        pss = ctx.enter_context(tc.tile_pool(name="pss", bufs=2, space="PSUM"))
        pso = ctx.enter_context(tc.tile_pool(name="pso", bufs=1, space="PSUM"))
        dram = ctx.enter_context(tc.tile_pool(name="dram", bufs=1, space="DRAM"))

        # ---- constants ----
        ident = const.tile([P, P], F32)
        make_identity(nc, ident)
        identb = const.tile([P, P], BF16)
        nc.scalar.copy(out=identb[:, :], in_=ident[:, :])
        eps_t = const.tile([P, 1], F32)
        nc.vector.memset(eps_t[:, :], EPS)
        ones01 = const.tile([1, P], BF16)
        nc.vector.memset(ones01[:, :], 0.1)       # bz stationary: bz = 0.1/Z
        ones512 = const.tile([1, 512], BF16)
        if not skip_bias:
            nc.vector.memset(ones512[:, :], 1.0)  # bias-broadcast moving row
        brow_sb = const.tile([1, HQ * HD + 2 * HKV * HD], BF16)
        msk_sb = const.tile([P, NT, 2, P], BF16)
        g2bc = const.tile([P, D], F32)
        b2bc = const.tile([P, D], F32)
        if not skip_g2b2:
            for row, dst in ((0, g2bc), (1, b2bc)):
                src = g2b2[row : row + 1, :]
                bsrc = bass.AP(tensor=src.tensor, offset=src.offset,
                               ap=[[0, P]] + src.ap[1:])
                nc.sync.dma_start(out=dst[:, :], in_=bsrc)

        wq_sb = const.tile([P, DC, HQ * HD], BF16)
        wkv_sb = const.tile([P, DC, 2 * HKV * HD], BF16)
        wo_sb = const.tile([P, NP, D], BF16)

        # ---- persistent tensors ----
        xtk = big.tile([P, 4, D], F32)            # raw tokens (residual)
        tT = big.tile([P, DC, 512], BF16)         # LN1(x)^T, own tokens
        qT = big.tile([HD, HQ, 512], BF16)        # per-head Q^T, base 0
        kstage = big.tile([HD, HKV, 512], BF16)   # own K^T (pre-AG)
        vtstage = big.tile([HD, HKV, 512], BF16)  # own V^T
        vnat = big.tile([P, 4, HKV, HD], BF16)    # own V natural (pre-AG)
        kvT = big.tile([HD, HKV, N], BF16)        # K^T, base partitions 0-63
        vt = big.tile([P, NT, HKV, 160], BF16)    # [V |1| pad31 | V]
        attnT = big.tile([P, NP, 512], BF16)      # 0.1 * normalized attn^T

        # ---- stage 1: LN1 + transpose (own 4 tiles) ----
        for t in range(4):
            nc.sync.dma_start(out=xtk[:, t, :], in_=xs[t, :, :])
        nc.sync.dma_start(out=wkv_sb[:, :, :], in_=wkv[:, :, :])
        nc.sync.dma_start(out=wq_sb[:, :, :], in_=wq[:, :, :])
        if not skip_bias:
            nc.sync.dma_start(out=brow_sb[:, :], in_=brow[:, :])
        nc.sync.dma_start(out=msk_sb[:, :, :, :], in_=msk[:, :, :, :])
        for t in range(4):
            xt = xtk[:, t, :]
            stats = stp.tile([P, 2, nc.vector.BN_STATS_DIM], F32, tag="bst")
            xg = xt.rearrange("p (s f) -> p s f", s=2)
            for s in range(2):
                nc.vector.bn_stats(out=stats[:, s, :], in_=xg[:, s, :])
            mv = stp.tile([P, nc.vector.BN_AGGR_DIM], F32, tag="mv")
            nc.vector.bn_aggr(out=mv[:, :], in_=stats[:, :, :])
            rstd = stp.tile([P, 1], F32, tag="rstd")
            nc.scalar.activation(out=rstd[:, :], in_=mv[:, 1:2], func=AF.Sqrt,
                                 bias=eps_t[:, :], scale=1.0)
            nc.vector.reciprocal(out=rstd[:, :], in_=rstd[:, :])
            xh = work.tile([P, D], BF16, tag="xh", bufs=2)
            nc.vector.tensor_scalar(out=xh[:, :], in0=xt,
                                    scalar1=mv[:, 0:1], scalar2=rstd[:, :],
                                    op0=ALU.subtract, op1=ALU.mult)
            for c in range(DC):
                pt = pssc.tile([P, P], BF16, tag="ps")
                nc.tensor.transpose(pt[:, :], xh[:, c * P : (c + 1) * P],
                                    identb[:, :])
                nc.vector.tensor_copy(out=tT[:, c, t * P : (t + 1) * P],
                                      in_=pt[:, :])

        # ---- stage 2a: KV proj (own tokens) -> AllGather ----
        for kv in range(HKV):
            ps = pssc.tile([P, 512], F32, tag="ps")
            for c in range(DC):
                last = (c == DC - 1) and skip_bias
                nc.tensor.matmul(ps[:, :], wkv_sb[:, c, kv * P : (kv + 1) * P],
                                 tT[:, c, :], start=(c == 0), stop=last)
            if not skip_bias:
                boff = HQ * HD + kv * P
                nc.tensor.matmul(ps[:, :], brow_sb[:, boff : boff + P],
                                 ones512[:, :], start=False, stop=True)
            nc.scalar.copy(out=kstage[:, kv, :], in_=ps[:HD, :])
            nc.scalar.copy(out=vtstage[:, kv, :], in_=ps[HD:P, :])
        for lt in range(4):
            pv = pssc.tile([P, HKV * HD], BF16, tag="ps")
            for kv in range(HKV):
                nc.tensor.transpose(pv[:, kv * HD : (kv + 1) * HD],
                                    vtstage[:, kv, lt * P : (lt + 1) * P],
                                    identb[:HD, :HD])
            nc.vector.tensor_copy(
                out=vnat[:, lt, :, :].rearrange("p k h -> p (k h)"),
                in_=pv[:, :])

        # one flat AllGather: per-rank slab = [K^T (64x4x512) | V (4x128x4x64)]
        HALF = HD * HKV * 512                       # 131072 elems per section
        ag_in = dram.tile([2 * HALF], BF16)
        ag_out = dram.tile([8 * HALF], BF16)
        agk_in = ag_in[0:HALF].rearrange("(h k t) -> h k t", h=HD, k=HKV)
        agv_in = ag_in[HALF : 2 * HALF].rearrange(
            "(l p k h) -> l p k h", l=4, p=P, k=HKV)
        nc.sync.dma_start(out=agk_in, in_=kstage[:, :, :])
        nc.sync.dma_start(out=agv_in.transpose([1, 0, 2, 3]),
                          in_=vnat[:, :, :, :])
        nc.gpsimd.collective_compute("AllGather", ALU.bypass,
                                     replica_groups=RG,
                                     ins=[ag_in[:]], outs=[ag_out[:]])

        # ---- stage 2b: Q proj (overlaps the AllGathers) ----
        # one 128-col psum covers a head pair; both head halves are copied
        # to base-partition-0 slots of qT so score matmuls never need the
        # upper PE row-group (whose packed form misbehaves on HW)
        for p in range(NP):
            ps = pssc.tile([P, 512], F32, tag="ps")
            for c in range(DC):
                last = (c == DC - 1) and skip_bias
                nc.tensor.matmul(ps[:, :], wq_sb[:, c, p * P : (p + 1) * P],
                                 tT[:, c, :], start=(c == 0), stop=last)
            if not skip_bias:
                nc.tensor.matmul(ps[:, :], brow_sb[:, p * P : (p + 1) * P],
                                 ones512[:, :], start=False, stop=True)
            nc.scalar.copy(out=qT[:, 2 * p, :], in_=ps[0:HD, :])
            qbnc = work.tile([HD, 512], BF16, tag="qbnc", bufs=2)
            nc.scalar.copy(out=qbnc[:, :], in_=ps[HD:P, :])
            nc.sync.dma_start(out=qT[:, 2 * p + 1, :], in_=qbnc[:, :])

        nc.sync.dma_start(out=wo_sb[:, :, :], in_=wo[:, :, :])

        # ---- land AG results ----
        nc.vector.memset(vt[:, :, :, 64:65], 1.0)
        nc.vector.memset(vt[:, :, :, 65:96], 0.0)
        for s in range(4):
            slab = ag_out[s * 2 * HALF : (s + 1) * 2 * HALF]
            kslab = slab[0:HALF].rearrange("(h k t) -> h k t", h=HD, k=HKV)
            vslab = slab[HALF : 2 * HALF].rearrange(
                "(l p k h) -> l p k h", l=4, p=P, k=HKV)
            for kv in range(HKV):
                nc.sync.dma_start(out=kvT[:, kv, s * 512 : (s + 1) * 512],
                                  in_=kslab[:, kv, :])
            for lt in range(4):
                pos = s * 4 + lt
                vsrc = vslab[lt, :, :, :]         # [P, HKV, HD]
                nc.sync.dma_start(out=vt[:, pos, :, 0:HD], in_=vsrc)
                nc.sync.dma_start(out=vt[:, pos, :, 96:160], in_=vsrc)

        # ---- stage 3: attention ----
        for p in range(NP):
            kv = p // 2
            psO_A = pso.tile([HD + 1, 512], F32, tag="poA")
            psO_B = pso.tile([P, 512], F32, tag="poB")
            for pos in range(NT):
                g = PERM[pos]
                qoff = P * (g // 4)
                w = 512 - qoff
                psS = pss.tile([P, 2, 512], F32, tag="ps")
                nc.tensor.matmul(psS[:, 0, qoff:512],
                                 kvT[:, kv, pos * P : (pos + 1) * P],
                                 qT[:, 2 * p, qoff:512],
                                 start=True, stop=True)
                nc.tensor.matmul(psS[:, 1, qoff:512],
                                 kvT[:, kv, pos * P : (pos + 1) * P],
                                 qT[:, 2 * p + 1, qoff:512],
                                 start=True, stop=True)
                uT = work.tile([P, 2, 512], BF16, tag="ut")
                sl = slice(qoff, 512)
                nc.scalar.activation(out=uT[:, :, sl], in_=psS[:, :, sl],
                                     func=AF.Exp, scale=SCALE)
                # rank-dependent causal 0/1 mask on the uncertain slot
                nc.vector.tensor_mul(uT[:, :, qoff : qoff + P],
                                     uT[:, :, qoff : qoff + P],
                                     msk_sb[:, pos, :, :])
                nc.tensor.matmul(psO_A[:, qoff:512], vt[:, pos, kv, 0:65],
                                 uT[:, 0, qoff:512],
                                 start=(pos == 0), stop=(pos == NT - 1))
                nc.tensor.matmul(psO_B[:, qoff:512],
                                 vt[:, pos, kv, 32:160],
                                 uT[:, 1, qoff:512],
                                 start=(pos == 0), stop=(pos == NT - 1))
            # normalize: attnT = psO * (0.1 / Z) per query column
            rzA = stp.tile([1, 512], F32, tag="rzA", bufs=2)
            rzB = stp.tile([1, 512], F32, tag="rzB", bufs=2)
            # custom-DVE reads PSUM unreliably on HW: stage Z rows in SBUF
            zA = stp.tile([1, 512], F32, tag="zA", bufs=2)
            zB = stp.tile([1, 512], F32, tag="zB", bufs=2)
            nc.scalar.copy(out=zA[:, :], in_=psO_A[HD : HD + 1, :])
            nc.scalar.copy(out=zB[:, :], in_=psO_B[32:33, :])
            nc.vector.reciprocal_approx_fast(out=rzA[:, :], in_=zA[:, :])
            nc.vector.reciprocal_approx_fast(out=rzB[:, :], in_=zB[:, :])
            rzAb = stp.tile([1, 512], BF16, tag="rzAb", bufs=2)
            rzBb = stp.tile([1, 512], BF16, tag="rzBb", bufs=2)
            nc.scalar.copy(out=rzAb[:, :], in_=rzA[:, :])
            nc.scalar.copy(out=rzBb[:, :], in_=rzB[:, :])
            bzA = pssc.tile([P, 512], F32, tag="ps")
            bzB = pssc.tile([P, 512], F32, tag="ps")
            nc.tensor.matmul(bzA[:, :], ones01[:, :], rzAb[:, :],
                             start=True, stop=True)
            nc.tensor.matmul(bzB[:, :], ones01[:, :], rzBb[:, :],
                             start=True, stop=True)
            bzA_sb = work.tile([P, 512], F32, tag="bzs", bufs=2)
            bzB_sb = work.tile([P, 512], F32, tag="bzs", bufs=2)
            nc.scalar.copy(out=bzA_sb[0:HD, :], in_=bzA[0:HD, :])
            nc.scalar.copy(out=bzB_sb[HD:P, :], in_=bzB[HD:P, :])
            nc.vector.tensor_mul(attnT[0:HD, p, :], psO_A[0:HD, :],
                                 bzA_sb[0:HD, :])
            nc.vector.tensor_mul(attnT[HD:P, p, :], psO_B[HD:P, :],
                                 bzB_sb[HD:P, :])

        # ---- stage 4: Wo + residual + LN2 ----
        for t in range(4):
            rt = work.tile([P, D], F32, tag="rt", bufs=2)
            for f in range(2):
                ps = pssc.tile([P, 512], F32, tag="ps")
                for p in range(NP):
                    nc.tensor.matmul(ps[:, :],
                                     attnT[:, p, t * P : (t + 1) * P],
                                     wo_sb[:, p, f * 512 : (f + 1) * 512],
                                     start=(p == 0), stop=(p == NP - 1))
                nc.vector.tensor_add(rt[:, f * 512 : (f + 1) * 512],
                                     ps[:, :],
                                     xtk[:, t, f * 512 : (f + 1) * 512])
            stats = stp.tile([P, 2, nc.vector.BN_STATS_DIM], F32, tag="bst")
            rg = rt.rearrange("p (s f) -> p s f", s=2)
            for s in range(2):
                nc.vector.bn_stats(out=stats[:, s, :], in_=rg[:, s, :])
            mv = stp.tile([P, nc.vector.BN_AGGR_DIM], F32, tag="mv")
            nc.vector.bn_aggr(out=mv[:, :], in_=stats[:, :, :])
            rstd = stp.tile([P, 1], F32, tag="rstd")
            nc.scalar.activation(out=rstd[:, :], in_=mv[:, 1:2], func=AF.Sqrt,
                                 bias=eps_t[:, :], scale=1.0)
            nc.vector.reciprocal(out=rstd[:, :], in_=rstd[:, :])
            yt = work.tile([P, D], F32, tag="yt", bufs=2)
            nc.vector.tensor_scalar(out=yt[:, :], in0=rt[:, :],
                                    scalar1=mv[:, 0:1], scalar2=rstd[:, :],
                                    op0=ALU.subtract, op1=ALU.mult)
            if not skip_g2b2:
                nc.vector.tensor_mul(yt[:, :], yt[:, :], g2bc[:, :])
                nc.vector.tensor_add(yt[:, :], yt[:, :], b2bc[:, :])
            nc.sync.dma_start(out=y[t, :, :], in_=yt[:, :])

    nc.finalize()
    return nc


_NC_CACHE = {}


def _get_program(skip_bias=False, skip_g2b2=False):
    key = (skip_bias, skip_g2b2)
    if key not in _NC_CACHE:
        _NC_CACHE[key] = build_program(*key)
    return _NC_CACHE[key]


def _bf(a):
    return np.ascontiguousarray(np.asarray(a, np.float32).astype(BF))


def make_in_maps(tokens, Wq, Wk, Wv, Wo, g1, b1, g2, b2):
    tokens = np.asarray(tokens, np.float32)
    g1 = np.asarray(g1, np.float32)
    b1 = np.asarray(b1, np.float32)
    # fold g1 into the projection weights; b1 contributes a constant row
    Wq_f = g1[:, None] * np.asarray(Wq, np.float32)
    Wk_f = g1[:, None] * np.asarray(Wk, np.float32)
    Wv_f = g1[:, None] * np.asarray(Wv, np.float32)
    # wkv columns: per kv head, [K_kv (64) | V_kv (64)] in 128-col blocks
    Wkv_f = np.empty((D, 2 * HKV * HD), np.float32)
    for kv in range(HKV):
        Wkv_f[:, kv * P : kv * P + HD] = Wk_f[:, kv * HD : (kv + 1) * HD]
        Wkv_f[:, kv * P + HD : (kv + 1) * P] = Wv_f[:, kv * HD : (kv + 1) * HD]
    bq = b1 @ Wq_f                      # [1024]
    bkv = b1 @ Wkv_f                    # [512]
    brow = np.concatenate([bq, bkv])[None, :]
    wq_r = Wq_f.reshape(DC, P, HQ * HD).transpose(1, 0, 2)
    wkv_r = Wkv_f.reshape(DC, P, 2 * HKV * HD).transpose(1, 0, 2)
    wo_r = np.asarray(Wo, np.float32).reshape(NP, P, D).transpose(1, 0, 2)
    g2b2 = np.stack([np.asarray(g2, np.float32), np.asarray(b2, np.float32)])

    tri = np.triu(np.ones((P, P), np.float32))   # keep iff key <= query
    in_maps = []
    for cid in range(8):
        b, r = cid // 4, cid % 4
        T = TILES[r]
        msk = np.empty((P, NT, 2, P), np.float32)
        for pos in range(NT):
            g = PERM[pos]
            own = T[g // 4]
            if g < own:
                m = np.ones((P, P), np.float32)
            elif g == own:
                m = tri
            else:
                m = np.zeros((P, P), np.float32)
            msk[:, pos, 0, :] = m
            msk[:, pos, 1, :] = m
        xs = np.ascontiguousarray(
            np.stack([tokens[b, g * P : (g + 1) * P] for g in T]))
        in_maps.append({
            "xs": xs,
            "wq": _bf(wq_r), "wkv": _bf(wkv_r), "wo": _bf(wo_r),
            "brow": _bf(brow), "g2b2": g2b2, "msk": _bf(msk),
        })
    return in_maps


def kernel(tokens, Wq, Wk, Wv, Wo, g1, b1, g2, b2, _trace=False,
           _trace_kwargs=None):
    skip_bias = bool(np.all(np.asarray(b1) == 0.0))
    skip_g2b2 = bool(np.all(np.asarray(g2) == 1.0)
                     and np.all(np.asarray(b2) == 0.0))
    nc = _get_program(skip_bias, skip_g2b2)
    in_maps = make_in_maps(tokens, Wq, Wk, Wv, Wo, g1, b1, g2, b2)
    res = run_bass_kernel_spmd(nc, in_maps, list(range(8)),
                               trace=_trace, **(_trace_kwargs or {}))
    out = np.empty((B, N, D), np.float32)
    for cid in range(8):
        b, r = cid // 4, cid % 4
        for t, g in enumerate(TILES[r]):
            out[b, g * P : (g + 1) * P] = res.results[cid]["y"][t]
    if _trace:
        return out, res
    return out
